# revision 5
# baseline (speedup 1.0000x reference)
"""Trainium2 Bass kernel for CapsNet DigitCaps dynamic routing (nn_DigitCaps).

Reference computation:
    u_hat[b,r,j,o] = W[r,j,o,:] @ x[b,r,:]        B,R,J,O,I = 512,1152,10,16,8
    b_ij = 0; 3 routing iterations:
        c = softmax(b_ij, axis=0)                  # over routes r, per j
        s[b,j,o] = sum_r c[r,j] * u_hat[b,r,j,o]
        v = squash(s) = s*|s|/(1+s^2)              # elementwise
        b_ij += mean_b sum_o u_hat[b,r,j,o]*v[b,j,o]
    return v[..., None]

Kernel strategy (data-parallel over batch, 8 cores, 64 rows each; u_hat is
never materialized):
    s    = X @ (c ⊙ W2)          X: [64, 9216]=[b,(r,i)], W2: [9216,160]=[(r,i),(j,o)]
    M    = X^T @ V               [9216, 160] per-core partial (batch outer product)
    bupd = sum_{i,o} W2 ⊙ M      computed as DVE product + PE block-ones matmuls
    bupd is AllReduce-summed across cores ([1152,10] = 46KB), softmax + weight
    scaling are computed redundantly on every core.
Matmuls run as float32r (~1 cyc/row at N>=256, rel err ~1.5e-4); everything
else is fp32.
"""
import os
import numpy as np
from contextlib import ExitStack

import concourse.bacc as bacc
import concourse.bass as bass
import concourse.tile as tile
from concourse import mybir
from concourse.bass_utils import run_bass_kernel_spmd

F32 = mybir.dt.float32
F32R = mybir.dt.float32r

B, R, J, O, I = 512, 1152, 10, 16, 8
N_CORES = 8
BL = B // N_CORES          # 64 batch rows per core
RI = R * I                 # 9216
NJO = J * O                # 160
NPAD = 256                 # padded matmul free dim (fp32r fast path needs >=256)
KT = RI // 128             # 72 contraction chunks
NUM_ITER = 3


def emit_algorithm(nc, tc, ctx, tensors, pools, out_d):
    """Emit one full 3-iteration routing computation."""
    (xT_s, x_s, W2_s, I8S_s, REPL_s, ONES_s, RONES_s) = tensors
    (sp, vq, wc_pool, p_pool, dram_pool,
     ps_s, ps_m, ps_bup, ps_crep, ps_small) = pools

    b_state = None
    for it in range(NUM_ITER):
        # ---- c preparation (softmax over r, replicated to (r,i) chunks) ----
        crep = None
        if it > 0:
            # b_state: [128, 90] layout [p, (rb, j)] with r = rb*128 + p
            e = sp.tile([128, 90], F32, tag="e")
            nc.scalar.activation(e[:], b_state[:], mybir.ActivationFunctionType.Exp)
            # column sums per j: ones-matmul -> [1, (j, rb)]
            sums_ps = ps_small.tile([1, 90], F32, tag="sums")
            nc.tensor.matmul(
                sums_ps[:].rearrange("p (j rb) -> p j rb", rb=9),
                ONES_s[:, 0:1],
                e[:].rearrange("p (rb j) -> p j rb", j=J),
                start=True, stop=True)
            ssum = sp.tile([1, 10], F32, tag="ssum")
            nc.vector.tensor_reduce(
                ssum[:], sums_ps[:].rearrange("p (j rb) -> p j rb", rb=9),
                axis=mybir.AxisListType.X, op=mybir.AluOpType.add)
            rcp = sp.tile([1, 10], F32, tag="rcp")
            nc.vector.reciprocal(rcp[:], ssum[:])
            # broadcast rcp over partitions: ones-outer-product matmul
            rcpb_ps = ps_small.tile([128, 10], F32, tag="rcpb")
            nc.tensor.matmul(rcpb_ps[:], RONES_s[:], rcp[:], start=True, stop=True)
            rcp_b = sp.tile([128, 10], F32, tag="rcp_b")
            nc.scalar.activation(rcp_b[:], rcpb_ps[:],
                                 mybir.ActivationFunctionType.Copy)
            cb = sp.tile([128, 90], F32, tag="cb")
            for rb in range(9):
                nc.vector.tensor_mul(cb[:, rb * 10:(rb + 1) * 10],
                                     e[:, rb * 10:(rb + 1) * 10], rcp_b[:])
            # replicate cb[r, j] -> crep[(r,i)-chunk partition layout, (k, j)]
            crep = sp.tile([128, 720], F32, tag="crep")
            for half, (m0, nm) in enumerate([(0, 5), (5, 4)]):
                cps = ps_crep.tile([128, 80 * nm], F32, tag="crep_ps")
                for g in range(8):
                    nc.tensor.matmul(
                        cps[:].rearrange("p (m x) -> p m x", x=80)[:, :, g * 10:g * 10 + 10],
                        REPL_s[:, g * 128:(g + 1) * 128],
                        cb[:, m0 * 10:(m0 + nm) * 10].rearrange("p (m j) -> p m j", j=J),
                        start=True, stop=True)
                nc.scalar.activation(crep[:, m0 * 80:(m0 + nm) * 80], cps[:],
                                     mybir.ActivationFunctionType.Copy)

        # ---- s matmul (+ weight scaling) ----
        s_ps = ps_s.tile([BL, NPAD], F32, tag="s")
        for k in range(KT):
            if it == 0:
                rhs = W2_s[:, k * NPAD:(k + 1) * NPAD]
            else:
                wc = wc_pool.tile([128, NPAD], F32R, tag="wc")
                in0 = W2_s[:, k * NPAD:k * NPAD + NJO].bitcast(F32) \
                    .rearrange("p (j o) -> p j o", o=O)
                in1 = crep[:, k * 10:(k + 1) * 10].rearrange("p (j o) -> p j o", o=1)
                i0b, i1b = bass.broadcast_tensor_aps(in0, in1)
                nc.vector.tensor_tensor(
                    wc[:, 0:NJO].rearrange("p (j o) -> p j o", o=O),
                    i0b, i1b, op=mybir.AluOpType.mult)
                rhs = wc[:]
            nc.tensor.matmul(s_ps[:], xT_s[:, k * BL:(k + 1) * BL], rhs,
                             start=(k == 0), stop=(k == KT - 1))

        # ---- squash ----
        kscl = (1.0 / R) if it == 0 else 1.0
        sabs = sp.tile([BL, NJO], F32, tag="sabs")
        nc.scalar.activation(sabs[:], s_ps[:, 0:NJO],
                             mybir.ActivationFunctionType.Abs, scale=kscl * kscl)
        den = sp.tile([BL, NJO], F32, tag="den")
        nc.scalar.activation(den[:], s_ps[:, 0:NJO],
                             mybir.ActivationFunctionType.Square, scale=kscl)
        den1 = sp.tile([BL, NJO], F32, tag="den1")
        nc.vector.tensor_scalar_add(den1[:], den[:], 1.0)
        rec = sp.tile([BL, NJO], F32, tag="rec")
        nc.vector.reciprocal(rec[:], den1[:])
        num = sp.tile([BL, NJO], F32, tag="num")
        nc.vector.tensor_mul(num[:], s_ps[:, 0:NJO], sabs[:])

        if it == NUM_ITER - 1:
            vout = vq.tile([BL, NJO], F32, tag="vout")
            nc.vector.tensor_mul(vout[:], num[:], rec[:])
            nc.sync.dma_start(out_d[:], vout[:])
            break

        vpad = vq.tile([BL, NPAD], F32R, tag="vpad")
        nc.vector.tensor_mul(vpad[:, 0:NJO], num[:], rec[:])

        # ---- M matmul + W2 contraction -> b_upd ----
        b_upd = sp.tile([128, 90], F32, tag="b_upd")
        for blk in range(9):
            bups = ps_bup.tile([128, NPAD], F32, tag="bup")
            for pair in range(4):
                mps = ps_m.tile([128, 2 * NPAD], F32, tag="m")
                for half in range(2):
                    c = blk * 8 + pair * 2 + half
                    nc.tensor.matmul(mps[:, half * NPAD:(half + 1) * NPAD],
                                     x_s[:, c * 128:(c + 1) * 128], vpad[:],
                                     start=True, stop=True)
                c0 = blk * 8 + pair * 2
                P = p_pool.tile([128, 2 * NPAD], F32R, tag="P")
                in0 = W2_s[:, c0 * NPAD:(c0 + 2) * NPAD].bitcast(F32) \
                    .rearrange("p (c n) -> p c n", n=NPAD)[:, :, 0:NJO]
                in1 = mps[:].rearrange("p (c n) -> p c n", n=NPAD)[:, :, 0:NJO]
                nc.vector.tensor_tensor(
                    P[:].rearrange("p (c n) -> p c n", n=NPAD)[:, :, 0:NJO],
                    in0, in1, op=mybir.AluOpType.mult)
                for half in range(2):
                    c = blk * 8 + pair * 2 + half
                    g = c % 8
                    nc.tensor.matmul(bups[:],
                                     I8S_s[:, g * 128:(g + 1) * 128],
                                     P[:, half * NPAD:(half + 1) * NPAD],
                                     start=(pair == 0 and half == 0),
                                     stop=(pair == 3 and half == 1))
            nc.vector.tensor_reduce(
                b_upd[:, blk * 10:(blk + 1) * 10],
                bups[:, 0:NJO].rearrange("p (j o) -> p j o", o=O),
                axis=mybir.AxisListType.X, op=mybir.AluOpType.add)

        # ---- AllReduce of b_upd ----
        cc_in = dram_pool.tile([128, 90], F32, tag="cc_in")
        cc_out = dram_pool.tile([128, 90], F32, tag="cc_out")
        nc.sync.dma_start(cc_in[:], b_upd[:])
        nc.gpsimd.collective_compute(
            "AllReduce", mybir.AluOpType.add,
            replica_groups=[list(range(N_CORES))],
            ins=[cc_in.opt()], outs=[cc_out.opt()])
        upd_g = sp.tile([128, 90], F32, tag=f"bstate{it}")
        nc.sync.dma_start(upd_g[:], cc_out[:])
        if it == 0:
            b_state = upd_g
        else:
            b2 = sp.tile([128, 90], F32, tag=f"bstate{it}b")
            nc.vector.tensor_add(b2[:], b_state[:], upd_g[:])
            b_state = b2


def build_nc(reps=1):
    nc = bacc.Bacc("TRN2", target_bir_lowering=False, debug=False,
                   num_devices=N_CORES)
    xT_d = nc.dram_tensor("xT", [RI, BL], F32R, kind="ExternalInput")
    x_d = nc.dram_tensor("x", [BL, RI], F32R, kind="ExternalInput")
    W2_d = nc.dram_tensor("W2", [RI, NJO], F32R, kind="ExternalInput")
    I8S_d = nc.dram_tensor("I8S", [128, 8 * 128], F32R, kind="ExternalInput")
    REPL_d = nc.dram_tensor("REPL", [128, 8 * 128], F32, kind="ExternalInput")
    ONES_d = nc.dram_tensor("ONES", [128, 1], F32, kind="ExternalInput")
    RONES_d = nc.dram_tensor("RONES", [1, 128], F32, kind="ExternalInput")
    out_d = nc.dram_tensor("out", [BL, NJO], F32, kind="ExternalOutput")

    with tile.TileContext(nc) as tc:
        with ExitStack() as ctx:
            pers = ctx.enter_context(tc.tile_pool(name="pers", bufs=1))
            sp = ctx.enter_context(tc.tile_pool(name="sp", bufs=2))
            vq = ctx.enter_context(tc.tile_pool(name="vq", bufs=2))
            wc_pool = ctx.enter_context(tc.tile_pool(name="wcp", bufs=6))
            p_pool = ctx.enter_context(tc.tile_pool(name="pp", bufs=3))
            dram_pool = ctx.enter_context(
                tc.tile_pool(name="dram", bufs=2, space="DRAM"))
            ps_s = ctx.enter_context(tc.tile_pool(name="ps_s", bufs=1, space="PSUM"))
            ps_m = ctx.enter_context(tc.tile_pool(name="ps_m", bufs=2, space="PSUM"))
            ps_bup = ctx.enter_context(tc.tile_pool(name="ps_b", bufs=2, space="PSUM"))
            ps_crep = ctx.enter_context(tc.tile_pool(name="ps_c", bufs=1, space="PSUM"))
            ps_small = ctx.enter_context(tc.tile_pool(name="ps_t", bufs=1, space="PSUM"))

            xT_s = pers.tile([128, KT * BL], F32R)
            x_s = pers.tile([BL, RI], F32R)
            W2_s = pers.tile([128, KT * NPAD], F32R)
            I8S_s = pers.tile([128, 8 * 128], F32R)
            REPL_s = pers.tile([128, 8 * 128], F32)
            ONES_s = pers.tile([128, 1], F32)
            RONES_s = pers.tile([1, 128], F32)

            # loads: W2/xT in 9 chunk-groups, x in 4 column groups
            for g in range(9):
                nc.sync.dma_start(
                    W2_s[:, g * 8 * NPAD:(g + 1) * 8 * NPAD]
                    .rearrange("p (c n) -> p c n", n=NPAD)[:, :, 0:NJO],
                    W2_d[:].rearrange("(c p) n -> p c n", p=128)[:, g * 8:(g + 1) * 8, :])
                nc.sync.dma_start(
                    xT_s[:, g * 8 * BL:(g + 1) * 8 * BL]
                    .rearrange("p (c m) -> p c m", m=BL),
                    xT_d[:].rearrange("(c p) m -> p c m", p=128)[:, g * 8:(g + 1) * 8, :])
            for g in range(4):
                nc.sync.dma_start(x_s[:, g * 2304:(g + 1) * 2304],
                                  x_d[:, g * 2304:(g + 1) * 2304])
            nc.sync.dma_start(I8S_s[:], I8S_d[:])
            nc.sync.dma_start(REPL_s[:], REPL_d[:])
            nc.sync.dma_start(ONES_s[:], ONES_d[:])
            nc.sync.dma_start(RONES_s[:], RONES_d[:])

            tensors = (xT_s, x_s, W2_s, I8S_s, REPL_s, ONES_s, RONES_s)
            pools = (sp, vq, wc_pool, p_pool, dram_pool,
                     ps_s, ps_m, ps_bup, ps_crep, ps_small)
            for rep in range(reps):
                emit_algorithm(nc, tc, ctx, tensors, pools, out_d)

    nc.compile()
    return nc


def make_host_inputs(x, W):
    """Build per-core in_maps from the full inputs."""
    x = np.ascontiguousarray(np.asarray(x, dtype=np.float32))
    W = np.asarray(W, dtype=np.float32)
    W2 = np.ascontiguousarray(W.transpose(0, 3, 1, 2).reshape(RI, NJO))

    I8S = np.zeros((128, 8 * 128), np.float32)
    for g in range(8):
        for m in range(16 * g, 16 * g + 16):
            q = m - 16 * g
            I8S[8 * q:8 * q + 8, g * 128 + m] = 1.0 / B
    REPL = np.zeros((128, 8 * 128), np.float32)
    for g in range(8):
        for m in range(128):
            REPL[16 * g + m // 8, g * 128 + m] = 1.0
    ONES = np.ones((128, 1), np.float32)
    RONES = np.ones((1, 128), np.float32)

    in_maps = []
    for c in range(N_CORES):
        xs = np.ascontiguousarray(x[c * BL:(c + 1) * BL].reshape(BL, RI))
        in_maps.append({
            "x": xs,
            "xT": np.ascontiguousarray(xs.T),
            "W2": W2,
            "I8S": I8S,
            "REPL": REPL,
            "ONES": ONES,
            "RONES": RONES,
        })
    return in_maps


def assemble_output(results):
    return np.concatenate(
        [results[c]["out"].reshape(BL, J, O, 1) for c in range(N_CORES)],
        axis=0).astype(np.float32)


_NC_CACHE = {}


def kernel(x, W):
    if "nc" not in _NC_CACHE:
        _NC_CACHE["nc"] = build_nc(reps=1)
    nc = _NC_CACHE["nc"]
    in_maps = make_host_inputs(x, W)
    res = run_bass_kernel_spmd(nc, in_maps, list(range(N_CORES)))
    return assemble_output(res.results)


if __name__ == "__main__":
    import reference
    inputs = reference.setup_inputs()
    expected = np.asarray(reference.reference(**inputs))
    got = kernel(np.asarray(inputs["x"]), np.asarray(inputs["W"]))
    err = np.abs(got - expected).max()
    rel = err / np.abs(expected).max()
    print("abs err:", err, "scale-rel err:", rel)
